# revision 11
# baseline (speedup 1.0000x reference)
# Trainium2 Bass kernel for nn_CTM_790273982469.
#
# Math: log_prob = s + mu + RHO * s @ theta_off.T  with  s = x @ beta.T
# Folding A = I + RHO * theta_off gives  log_prob = s @ A.T + mu.
#
# Sharding: the contraction (vocab) dim V=50000 is split across 8 cores
# (6250 each).  Each core computes a partial  s_c.T = beta_cT.T-style
# accumulation on the tensor engine and emits  lp_c = s_c @ A.T + mu/8;
# the host gather is a sum of the 8 partials.
#
# Per-core device program (fp32 throughout):
#   - x arrives pre-transposed ([V_c, B], contiguous) so v-chunks of 128
#     land on SBUF partitions with unit-stride DMAs.
#   - For each 128-wide v-chunk: matmul(psum_sT, lhsT=betaT_chunk[128,64],
#     rhs=xT_chunk[128,512-slice]) accumulating sT = s.T in PSUM.
#     Even/odd chunks go to PE column halves 0-63 / 64-127 (col tiling),
#     which both doubles PE throughput and stacks the two partial sT
#     halves on PSUM partitions 0-63 / 64-127.
#   - Epilogue: one matmul per 128-row output block with
#     lhsT = sT[:, block] (128x128) and rhs = [A.T; A.T] (128x64) folds
#     the even+odd halves and applies A in one shot; DVE adds mu/8.

import numpy as np

P = 128
B_FULL = 2048
V_FULL = 50000
K = 64
RHO = 0.1
N_CORES = 8
VP_FULL = V_FULL // N_CORES  # 6250
MM_N = 512        # moving free-dim per matmul (fp32 max)
DMA_PAIR = 4      # full v-chunks per x DMA (4 MB transfers)


def _build_nc(b=B_FULL, vp=VP_FULL, col_pack=True):
    import concourse.bacc as bacc
    import concourse.mybir as mybir
    import concourse.tile as tile

    f32 = mybir.dt.float32
    nch = (vp + P - 1) // P          # v-chunks per core (last may be short)
    nfull = vp // P                  # full 128-row chunks
    rem = vp - nfull * P             # rows in the short chunk (0 if none)
    nbs = (b + MM_N - 1) // MM_N     # 512-wide b slices
    nbb = b // P                     # 128-row output blocks

    nc = bacc.Bacc()
    xt = nc.declare_dram_parameter("xt", [vp, b], f32, isOutput=False)
    betata = nc.declare_dram_parameter("betata", [P, nch * K], f32, isOutput=False)
    atst = nc.declare_dram_parameter("atst", [P, K], f32, isOutput=False)
    mu8 = nc.declare_dram_parameter("mu8", [P, K], f32, isOutput=False)
    out = nc.declare_dram_parameter("out", [b, K], f32, isOutput=True)

    # Even chunks accumulate on PE column-half 0 -> psum partitions 0-63,
    # banks 0-3 (free cols 0:b).  Odd chunks -> partitions 64-127, banks 4-7
    # (free cols b:2b).  Disjoint banks keep the two accumulation groups'
    # zero regions independent; disjoint column groups let the two matmul
    # streams run concurrently on the PE array.
    if col_pack:
        halves = [list(range(0, nch, 2)), list(range(1, nch, 2))]
    else:
        halves = [list(range(nch))]
    half_w = ((b + MM_N - 1) // MM_N) * MM_N  # per-half psum width, bank multiple
    poff, boff, first, last = {}, {}, {}, {}
    for hi, h in enumerate(halves):
        for c in h:
            poff[c] = hi * K if col_pack else 0
            boff[c] = hi * half_w if col_pack else 0
            first[c] = c == h[0]
            last[c] = c == h[-1]

    with tile.TileContext(nc) as tc:
        with (
            tc.tile_pool(name="const", bufs=1) as cpool,
            tc.tile_pool(name="xin", bufs=3) as xpool,
            tc.tile_pool(name="work", bufs=1) as wpool,
            tc.tile_pool(name="psacc", bufs=1, space="PSUM") as psacc,
            tc.tile_pool(name="pslp", bufs=2, space="PSUM") as pslp,
        ):
            beta_sb = cpool.tile([P, nch * K], f32)
            nc.sync.dma_start(beta_sb[:], betata[:])
            atst_sb = cpool.tile([P, K], f32)
            nc.sync.dma_start(atst_sb[:], atst[:])
            mu8_sb = cpool.tile([P, K], f32)
            nc.sync.dma_start(mu8_sb[:], mu8[:])

            acc_w = 2 * half_w if col_pack else b
            ps_sT = psacc.tile([P, acc_w], f32, tag="ps")  # sT accumulator

            def mm_chunk_slice(c, xt_ap, s):
                ns = min(MM_N, b - s * MM_N)
                nc.tensor.matmul(
                    ps_sT[
                        poff[c] : poff[c] + K,
                        boff[c] + s * MM_N : boff[c] + s * MM_N + ns,
                    ],
                    beta_sb[:, c * K : (c + 1) * K],
                    xt_ap[:, s * MM_N : s * MM_N + ns],
                    start=first[c],
                    stop=last[c],
                )

            def do_chunks(chunks_and_aps):
                # slice-major interleave so matmuls alternate PE column halves
                for s in range(nbs):
                    for c, xt_ap in chunks_and_aps:
                        mm_chunk_slice(c, xt_ap, s)

            for cp in range(0, nfull, DMA_PAIR):
                npair = min(DMA_PAIR, nfull - cp)
                xt_sb = xpool.tile([P, DMA_PAIR, b], f32, tag="xt")
                nc.sync.dma_start(
                    xt_sb[:, :npair, :],
                    xt[cp * P : (cp + npair) * P, :].rearrange(
                        "(c p) b -> p c b", p=P
                    ),
                )
                for i in range(0, npair, 2):
                    do_chunks(
                        [
                            (cp + i + j, xt_sb[:, i + j, :])
                            for j in range(min(2, npair - i))
                        ]
                    )

            if rem:
                xr_sb = xpool.tile([P, DMA_PAIR, b], f32, tag="xt")
                nc.any.memzero(xr_sb[:, 0, :])
                nc.sync.dma_start(xr_sb[:rem, 0, :], xt[nfull * P :, :])
                do_chunks([(nfull, xr_sb[:, 0, :])])

            sT_sb = wpool.tile([P, b], f32)
            if col_pack:
                nc.vector.tensor_copy(out=sT_sb[:K, :], in_=ps_sT[:K, :b])
                nc.vector.tensor_copy(
                    out=sT_sb[K:P, :], in_=ps_sT[K:P, half_w : half_w + b]
                )
            else:
                nc.any.memzero(sT_sb[K:P, :])
                nc.vector.tensor_copy(out=sT_sb[:K, :], in_=ps_sT[:K, :])

            out_sb = wpool.tile([P, nbb, K], f32)
            for bi in range(nbb):
                # psum tiles must stay exact bank multiples (512 f32) so the
                # pool's slot packing keeps every tile bank-aligned
                if col_pack:
                    ps_lp = psacc.tile([P, MM_N], f32, tag="ps")
                else:
                    ps_lp = pslp.tile([P, MM_N], f32, tag="lp")
                nc.tensor.matmul(
                    ps_lp[:, :K],
                    sT_sb[:, bi * P : (bi + 1) * P],
                    atst_sb[:],
                    start=True,
                    stop=True,
                )
                nc.vector.tensor_add(
                    out=out_sb[:, bi, :], in0=ps_lp[:, :K], in1=mu8_sb[:]
                )
            nc.sync.dma_start(
                out.rearrange("(n p) k -> p n k", p=P), out_sb[:]
            )
    if not nc.is_finalized():
        nc.finalize()
    return nc


def _host_prep(x, beta, theta, mu, n_cores=N_CORES):
    """Shard + lay out inputs for the per-core device program."""
    b = x.shape[0]
    v = x.shape[1]
    vp = v // n_cores
    nch = (vp + P - 1) // P

    xT = np.ascontiguousarray(x.T.astype(np.float32, copy=False))  # [V, B]

    eye = np.eye(K, dtype=np.float32)
    a_mat = eye + np.float32(RHO) * (theta.astype(np.float32) * (1.0 - eye))
    atst = np.ascontiguousarray(
        np.concatenate([a_mat.T, a_mat.T], axis=0).astype(np.float32)
    )  # [128, 64]
    mu8 = np.ascontiguousarray(
        np.tile((mu.astype(np.float32) / np.float32(n_cores))[None, :], (P, 1))
    )  # [128, 64]

    in_maps = []
    for c in range(n_cores):
        bt = beta[:, c * vp : (c + 1) * vp].T.astype(np.float32)  # [vp, 64]
        arr = np.zeros((nch * P, K), np.float32)
        arr[:vp] = bt
        betata = np.ascontiguousarray(
            arr.reshape(nch, P, K).transpose(1, 0, 2).reshape(P, nch * K)
        )
        in_maps.append(
            {
                "xt": np.ascontiguousarray(xT[c * vp : (c + 1) * vp]),
                "betata": betata,
                "atst": atst,
                "mu8": mu8,
            }
        )
    return in_maps


def kernel(x, beta, theta, mu):
    from concourse.bass_utils import run_bass_kernel_spmd

    in_maps = _host_prep(x, beta, theta, mu)
    nc = _build_nc()
    res = run_bass_kernel_spmd(nc, in_maps, list(range(N_CORES)))
    parts = np.stack([res.results[i]["out"] for i in range(N_CORES)])
    return parts.sum(axis=0).astype(np.float32)
